# revision 10
# baseline (speedup 1.0000x reference)
"""JPEG compression kernel for Trainium2, 8-core data-parallel.

Algorithm (per 8x8 block, per batch image):
  yuv = R1 @ rgb ; d = M (255*yuv-128) M^T ; t = round(d/Q) ; r = M^T (Q*t) M
  out = (R2 @ r + 128)/255
The -128 / +128 shifts cancel exactly through the DC coefficient
(round(e - 64) == round(e) - 64 for the integer 64 = 128*8*sqrt(2)*M00^2/Q00),
so the kernel computes out = A2 @ (Q * round(A1 @ x)) with the color
matrices, the 255 / (1/255) scales and 1/Q folded into the matmul weights.

On-chip pipeline per [96,512] tile (96 = 3 channels x 4 row-blocks x 8 rows,
512 = image width), all matmuls in float32r (TF32-like, 1 cyc/row):
  DMA-in -> Vf (vertical DCT + rgb2yuv, PE) -> evac (ACT) -> T1 (PE transpose)
  -> evac (ACT) -> Hf (horizontal DCT, PE) -> quant (DVE tt x 1/Q)
  -> round (magic-number tensor_scalar) -> dequant (tt x Q, f32r out)
  -> Hi (PE) -> evac (ACT) -> T2 (PE transpose) -> evac -> Vi (+yuv2rgb, PE)
  -> final evac -> DMA-out.

Self-contained: only needs numpy + the concourse/bass runtime installed in
the environment (import path /opt/trn_rl_repo).
"""

import sys

for _p in ("/opt/trn_rl_repo", "/root/.axon_site/_ro/trn_rl_repo"):
    if _p not in sys.path:
        sys.path.insert(0, _p)

import numpy as np
from contextlib import ExitStack

import concourse.bass as bass
import concourse.tile as tile
from concourse import bacc, mybir
from concourse.bass_utils import run_bass_kernel_spmd

F32 = mybir.dt.float32
F32R = mybir.dt.float32r

N_CORES = 8
B_PER_CORE = 2          # 16 images / 8 cores
H = W = 512
G = 4                   # row-blocks per tile -> 32 rows
PV = 96                 # 3 ch * G * 8
N_STRIPE = H // (8 * G)  # 16 row-stripes per image
MAGIC = float(np.float32(1.5 * 2**23))

_nc_cache = {}


def _build_weights(rgb2yuv, yuv2rgb, dct_coeff, q_lum, q_chrom):
    """Host-side weight prep (all tiny). Returns dict of np arrays."""
    C = np.asarray(dct_coeff, np.float64)
    M = C[:, 0, :, 0] / np.sqrt(C[0, 0, 0, 0])      # (8,8) 1-D DCT basis
    R1 = np.asarray(rgb2yuv, np.float64)
    R2 = np.asarray(yuv2rgb, np.float64)
    Q = np.stack([np.asarray(q_lum, np.float64),
                  np.asarray(q_chrom, np.float64),
                  np.asarray(q_chrom, np.float64)])  # (3,8,8)[i][u,v]

    # NOTE: the reference's "inverse" einsum applies the basis array in the
    # SAME orientation as the forward (r = A @ dq @ A.T with A = M as an
    # array), so the inverse weights use A[x,u] / A[y,v], i.e. M.T blocks.
    # Vf: k=(c,g,x) -> m=(i,g,u) : 255*R1[i,c]*M[u,x]
    W1 = np.zeros((PV, PV))
    # Vi: k=(i,g,u) -> m=(c,g,x) : R2[c,i]*A[x,u]/255
    W4 = np.zeros((PV, PV))
    for g in range(G):
        for a in range(3):
            for b in range(3):
                ka = slice(a * 32 + g * 8, a * 32 + g * 8 + 8)
                kb = slice(b * 32 + g * 8, b * 32 + g * 8 + 8)
                # W1[k=(c=a,g,x), m=(i=b,g,u)] = 255*R1[b,a]*M[u,x] -> [x,u] = M.T
                W1[ka, kb] += 255.0 * R1[b, a] * M.T
                # W4[k=(i=a,g,u), m=(c=b,g,x)] = R2[b,a]*A[x,u]/255 -> [u,x] = M.T
                W4[ka, kb] += R2[b, a] * M.T / 255.0

    # Hf: k=(j,y) -> m=(j,v) : M[v,y] ; Hi: k=(j,v) -> m=(j,y) : A[y,v]=M[y,v]
    WH = np.zeros((128, 128))
    WHi = np.zeros((128, 128))
    for j in range(16):
        s = slice(j * 8, j * 8 + 8)
        WH[s, s] = M.T          # [y,v] = M[v,y] -> M.T
        WHi[s, s] = M.T         # [v,y] = A[y,v] = M[y,v] -> M.T

    # quant tables at coeff layout [part=(j,v), free=(i,g,u)]
    invQt = np.zeros((128, PV))
    Qt = np.zeros((128, PV))
    for j in range(16):
        for v in range(8):
            p = j * 8 + v
            for i in range(3):
                for g in range(G):
                    fs = i * 32 + g * 8
                    invQt[p, fs:fs + 8] = 1.0 / Q[i, :, v]
                    Qt[p, fs:fs + 8] = Q[i, :, v]

    # The host pre-centers the input (x - 128/255), which covers the -128
    # shift for the Y channel (rgb2yuv row 0 sums to 1; rows 1,2 sum to 0).
    # Residual: -128*sum_x(M[u,x]) on u=0 partitions of U,V after Vf.
    # Output side: +128 per yuv channel maps to +128*rowsum(R2)/255 per rgb
    # output channel after Vi.
    s8 = M.sum(axis=1)
    vfbias = np.zeros((PV, 1))
    obias = np.zeros((PV, 1))
    for g in range(G):
        for a in range(3):
            if a > 0:
                vfbias[a * 32 + g * 8, 0] = -128.0 * s8[0]
            obias[a * 32 + g * 8:a * 32 + g * 8 + 8, 0] = \
                128.0 * R2[a].sum() / 255.0

    return {
        "vfbias": np.ascontiguousarray(vfbias, np.float32),
        "obias": np.ascontiguousarray(obias, np.float32),
        "w1": np.ascontiguousarray(W1, np.float32),
        "wh": np.ascontiguousarray(WH, np.float32),
        "whi": np.ascontiguousarray(WHi, np.float32),
        "w4": np.ascontiguousarray(W4, np.float32),
        "invqt": np.ascontiguousarray(invQt, np.float32),
        "qt": np.ascontiguousarray(Qt, np.float32),
        "ident": np.eye(128, dtype=np.float32),
    }


def _build_program():
    nc = bacc.Bacc("TRN2", target_bir_lowering=False, debug=False)

    x_d = nc.dram_tensor("x", [B_PER_CORE, 3, N_STRIPE, 8 * G, W], F32R,
                         kind="ExternalInput")
    w1_d = nc.dram_tensor("w1", [PV, PV], F32R, kind="ExternalInput")
    wh_d = nc.dram_tensor("wh", [128, 128], F32R, kind="ExternalInput")
    whi_d = nc.dram_tensor("whi", [128, 128], F32R, kind="ExternalInput")
    w4_d = nc.dram_tensor("w4", [PV, PV], F32R, kind="ExternalInput")
    iq_d = nc.dram_tensor("invqt", [128, PV], F32, kind="ExternalInput")
    qt_d = nc.dram_tensor("qt", [128, PV], F32, kind="ExternalInput")
    id_d = nc.dram_tensor("ident", [128, 128], F32R, kind="ExternalInput")
    vb_d = nc.dram_tensor("vfbias", [PV, 1], F32, kind="ExternalInput")
    ob_d = nc.dram_tensor("obias", [PV, 1], F32, kind="ExternalInput")
    o_d = nc.dram_tensor("out", [B_PER_CORE, 3, N_STRIPE, 8 * G, W], F32,
                         kind="ExternalOutput")

    FH = 4 * PV   # H-domain tile free size (4 w-chunks of 96)

    with tile.TileContext(nc) as tc, ExitStack() as ctx:
        wpool = ctx.enter_context(tc.tile_pool(name="wts", bufs=1))
        inp = ctx.enter_context(tc.tile_pool(name="inp", bufs=3))
        sb = ctx.enter_context(tc.tile_pool(name="sb", bufs=2))
        outp = ctx.enter_context(tc.tile_pool(name="outp", bufs=3))
        ps = ctx.enter_context(tc.tile_pool(name="ps", bufs=1, space="PSUM"))
        ps2 = ctx.enter_context(tc.tile_pool(name="ps2", bufs=1, space="PSUM"))

        w1_t = wpool.tile([PV, PV], F32R)
        nc.sync.dma_start(w1_t[:], w1_d[:])
        wh_t = wpool.tile([128, 128], F32R)
        nc.sync.dma_start(wh_t[:], wh_d[:])
        whi_t = wpool.tile([128, 128], F32R)
        nc.sync.dma_start(whi_t[:], whi_d[:])
        w4_t = wpool.tile([PV, PV], F32R)
        nc.sync.dma_start(w4_t[:], w4_d[:])
        iq_t = wpool.tile([128, PV], F32)
        nc.sync.dma_start(iq_t[:], iq_d[:])
        qt_t = wpool.tile([128, PV], F32)
        nc.sync.dma_start(qt_t[:], qt_d[:])
        id_t = wpool.tile([128, 128], F32R)
        nc.sync.dma_start(id_t[:], id_d[:])
        vb_t = wpool.tile([PV, 1], F32)
        nc.sync.dma_start(vb_t[:], vb_d[:])
        ob_t = wpool.tile([PV, 1], F32)
        nc.sync.dma_start(ob_t[:], ob_d[:])

        for b in range(B_PER_CORE):
            for t in range(N_STRIPE):
                in_t = inp.tile([PV, W], F32R, tag="in")
                nc.sync.dma_start(in_t[:], x_d[b, :, t])

                # V-forward (vertical DCT + rgb2yuv + 255 scale)
                vf_ps = ps.tile([PV, W], F32, tag="vf")
                nc.tensor.matmul(vf_ps[:], w1_t[:],
                                 in_t[:].bitcast(F32R), start=True, stop=True)
                vf_sb = sb.tile([PV, W], F32R, tag="vfsb")
                nc.vector.tensor_scalar_add(vf_sb[:], vf_ps[:], vb_t[:])

                # T1: 4x PE transpose [96,128] -> [128,96]
                t1_ps = ps.tile([128, FH], F32R, tag="t1")
                for q in range(4):
                    nc.tensor.transpose(
                        t1_ps[:, q * PV:(q + 1) * PV],
                        vf_sb[:, q * 128:(q + 1) * 128],
                        id_t[0:PV, 0:PV],
                    )
                hf_in = sb.tile([128, FH], F32R, tag="hfin")
                nc.scalar.copy(hf_in[:], t1_ps[:])

                # H-forward
                hf_ps = ps.tile([128, FH], F32, tag="hf")
                nc.tensor.matmul(hf_ps[:], wh_t[:], hf_in[:],
                                 start=True, stop=True)

                # quantize: e = d * (1/Q)   (DVE, PSUM source)
                e_t = sb.tile([128, FH], F32, tag="e")
                nc.vector.tensor_tensor(
                    e_t[:].rearrange("p (r k) -> p r k", k=PV),
                    hf_ps[:].rearrange("p (r k) -> p r k", k=PV),
                    iq_t[:].unsqueeze(1).broadcast_to((128, 4, PV)),
                    mybir.AluOpType.mult,
                )
                # round to nearest-even via magic constant
                t_t = sb.tile([128, FH], F32, tag="t")
                nc.vector.tensor_scalar(
                    t_t[:], e_t[:], MAGIC, MAGIC,
                    mybir.AluOpType.add, mybir.AluOpType.subtract,
                )
                # dequantize: r = t * Q  (f32r out, feeds H-inverse)
                r_t = sb.tile([128, FH], F32R, tag="r")
                nc.gpsimd.tensor_tensor(
                    r_t[:].rearrange("p (r k) -> p r k", k=PV),
                    t_t[:].rearrange("p (r k) -> p r k", k=PV),
                    qt_t[:].unsqueeze(1).broadcast_to((128, 4, PV)),
                    mybir.AluOpType.mult,
                )

                # H-inverse
                hi_ps = ps2.tile([128, FH], F32, tag="hi")
                nc.tensor.matmul(hi_ps[:], whi_t[:], r_t[:],
                                 start=True, stop=True)
                hi_sb = sb.tile([128, FH], F32R, tag="hisb")
                nc.scalar.copy(hi_sb[:], hi_ps[:])

                # T2: 4x PE transpose [128,96] -> [96,128]
                t2_ps = ps2.tile([PV, W], F32R, tag="t2")
                for q in range(4):
                    nc.tensor.transpose(
                        t2_ps[:, q * 128:(q + 1) * 128],
                        hi_sb[:, q * PV:(q + 1) * PV],
                        id_t[:],
                    )
                vi_in = sb.tile([PV, W], F32R, tag="viin")
                nc.scalar.copy(vi_in[:], t2_ps[:])

                # V-inverse (+ yuv2rgb + 1/255)
                vi_ps = ps2.tile([PV, W], F32, tag="vi")
                nc.tensor.matmul(vi_ps[:], w4_t[:], vi_in[:],
                                 start=True, stop=True)
                out_sb = outp.tile([PV, W], F32, tag="out")
                nc.vector.tensor_scalar_add(out_sb[:], vi_ps[:], ob_t[:])

                nc.sync.dma_start(o_d[b, :, t], out_sb[:])

    nc.compile()
    return nc


def make_in_maps(image, rgb2yuv, yuv2rgb, dct_coeff, q_lum, q_chrom):
    image = np.asarray(image, np.float32)
    wts = _build_weights(rgb2yuv, yuv2rgb, dct_coeff, q_lum, q_chrom)
    in_maps = []
    image = image - np.float32(128.0 / 255.0)  # center (see _build_weights)
    for core in range(N_CORES):
        shard = image[core * B_PER_CORE:(core + 1) * B_PER_CORE]
        shard = np.ascontiguousarray(shard).reshape(
            B_PER_CORE, 3, N_STRIPE, 8 * G, W)
        m = {"x": shard}
        m.update(wts)
        in_maps.append(m)
    return in_maps


def kernel(image, rgb2yuv, yuv2rgb, dct_coeff, q_lum, q_chrom):
    nb = np.asarray(image).shape[0]
    in_maps = make_in_maps(image, rgb2yuv, yuv2rgb, dct_coeff, q_lum, q_chrom)

    if "nc" not in _nc_cache:
        _nc_cache["nc"] = _build_program()
    nc = _nc_cache["nc"]

    res = run_bass_kernel_spmd(nc, in_maps, list(range(N_CORES)))
    outs = [res.results[c]["out"].reshape(B_PER_CORE, 3, H, W)
            for c in range(N_CORES)]
    return np.concatenate(outs, axis=0)[:nb]


if __name__ == "__main__":
    rng = np.random.default_rng(0)
    img = rng.uniform(size=(16, 3, 512, 512)).astype(np.float32)
    print(kernel(img,
                 np.array([[0.299, 0.587, 0.114],
                           [-0.1687, -0.3313, 0.5],
                           [0.5, -0.4187, -0.0813]], np.float32),
                 np.array([[1.0, 0.0, 1.402],
                           [1.0, -0.34414, -0.71414],
                           [1.0, 1.772, 0.0]], np.float32),
                 None, None, None).shape)
